# revision 28
# baseline (speedup 1.0000x reference)
"""Trainium2 Bass kernel for nn_MessagePassing (gnn_message_passing).

Reference computation (2 steps):
    h    = relu(cur @ mW1 + mb1)                      # per-module MLP layer 1
    msg  = h @ mW2 + mb2                              # per-module MLP layer 2
    rec  = einsum('mn,bnd->bmd', C, msg) * w[:,:,None]
    g    = relu(concat([cur, rec], -1) @ aW1 + ab1)
    cur  = cur + g @ aW2 + ab2

Strategy (data-parallel over 8 NeuronCores, 8192 batch rows each):
  * T-layout on chip: SBUF tiles are [128 features (partitions), cols] where
    a column is one (b, m) row of the flattened [B*M, 128] stream. Every
    per-module linear layer is one weights-stationary bf16 matmul
    (out = W.T @ x_T) streaming 512 columns per instruction; PSUM fp32;
    the fp32 residual stream never passes through bf16.
  * Algebraic refactor (host-side): row scaling by w commutes with
    right-matmuls and the 8x8 mix commutes with feature transforms:
        rec_contrib = (w ⊙ mix(h)) @ Q + s ⊗ qb
    with Q = mW2 @ aW1_bot, qb = mb2 @ aW1_bot, s = w * rowsum(C). This
    eliminates the mW2 pass. The rank-1 s⊗qb term is one K=32 matmul per
    group against a host-built selector table. ab2 is deferred to the host
    (+NSTEPS*ab2 at the end); step-1's bf16 operand re-adds it on-device.
  * The module mix runs on the PE as ONE weights-stationary matmul with
    W_mix = kron(I_16, C.T) (block-diagonal over 16 b-groups x 8 modules).
    Rows<->partitions movement uses the DMA XBAR sub-tiled transpose
    ([128,(t,d)] <-> [d,(t,128)]), batched 4 groups (2048 cols) per DMA
    instruction to amortize the ~600ns trigger + ~20ns/16x128-tile
    descriptor cost.
  * DRAM I/O is quad-blocked ([nq, 128, 2048] contiguous 1MB blocks; the
    flat layout cost 4.1us of descriptor generation per DMA).
  * Engine balance per group-step: PE 6 matmuls; ACT h-relu evac + trB
    trigger; DVE g-relu evac + residual + w-scale; SP loads/stores + trA
    trigger; GPSIMD the step-1 bf16 recast. The w-scale is one
    tensor_tensor against a 0-stride broadcast of per-tile w columns.
  * Emission is software-pipelined over super-groups of 2 quads: all
    front-half work (h matmul, relu, trA, mix, w-scale, trB) for a quad is
    emitted before its back-half (aW1/rank1 accumulate, relu, aW2,
    residual), so the PE's in-order stream never waits on the transpose
    round-trip.
"""

import os
import sys

import numpy as np

try:
    import concourse.bass as bass
except ImportError:  # harness runs kernel.py from a bare directory
    sys.path.insert(0, "/opt/trn_rl_repo")
    import concourse.bass as bass

import ml_dtypes
import concourse.bacc as bacc
import concourse.mybir as mybir
from concourse.tile import TileContext

BF16 = ml_dtypes.bfloat16
D = 128
M = 8
GRP = 512
QB = 4                  # groups per quad (DMA/transpose batch)
QCOLS = QB * GRP        # 2048
SGQ = 6                 # quads per super-group (software pipeline depth)
NCORES = 8
NSTEPS = 2

_nc_cache = {}


def build_nc(cols):
    """Build (and cache) the per-core Bass program for a `cols`-wide shard."""
    if cols in _nc_cache:
        return _nc_cache[cols]
    assert cols % QCOLS == 0
    ng = cols // GRP
    nq = cols // QCOLS
    sfree = ((ng + 31) // 32) * GRP

    f32 = mybir.dt.float32
    bf = mybir.dt.bfloat16
    relu = mybir.ActivationFunctionType.Relu
    add = mybir.AluOpType.add
    mult = mybir.AluOpType.mult
    amax = mybir.AluOpType.max

    nc = bacc.Bacc(trn_type="TRN2")
    xb_d = nc.declare_dram_parameter("xb", [nq, D, QCOLS], bf, isOutput=False)
    wcol_d = nc.declare_dram_parameter("wcol", [D, 4 * ng], f32, isOutput=False)
    s32_d = nc.declare_dram_parameter("s32", [D, sfree], bf, isOutput=False)
    qb32_d = nc.declare_dram_parameter("qb32", [D, 32 * D], bf, isOutput=False)
    wm1_d = nc.declare_dram_parameter("wm1", [D, D], bf, isOutput=False)
    wmx_d = nc.declare_dram_parameter("wmx", [D, D], bf, isOutput=False)
    wq_d = nc.declare_dram_parameter("wq", [D, D], bf, isOutput=False)
    wa1t_d = nc.declare_dram_parameter("wa1t", [D, D], bf, isOutput=False)
    wa2_d = nc.declare_dram_parameter("wa2", [D, D], bf, isOutput=False)
    mb1_d = nc.declare_dram_parameter("mb1", [D, 1], f32, isOutput=False)
    ab1_d = nc.declare_dram_parameter("ab1", [D, 1], f32, isOutput=False)
    ab2_d = nc.declare_dram_parameter("ab2", [D, 1], f32, isOutput=False)
    out_d = nc.declare_dram_parameter("out", [nq, D, QCOLS], f32, isOutput=True)

    nsq = (nq + SGQ - 1) // SGQ

    with TileContext(nc) as tc:
        with (
            tc.tile_pool(name="consts", bufs=1) as cp,
            tc.tile_pool(name="work", bufs=2) as wp,
            tc.tile_pool(name="pipe", bufs=SGQ + 1) as fp,
            tc.tile_pool(name="psum", bufs=2, space="PSUM") as pp,
        ):
            w_m1 = cp.tile_from(forced_dma_engine=mybir.EngineType.Pool, ap=wm1_d[:, :])
            w_mx = cp.tile_from(forced_dma_engine=mybir.EngineType.Pool, ap=wmx_d[:, :])
            w_q = cp.tile_from(forced_dma_engine=mybir.EngineType.Pool, ap=wq_d[:, :])
            w_a1t = cp.tile_from(forced_dma_engine=mybir.EngineType.Pool, ap=wa1t_d[:, :])
            w_a2 = cp.tile_from(forced_dma_engine=mybir.EngineType.Pool, ap=wa2_d[:, :])
            qb32 = cp.tile_from(forced_dma_engine=mybir.EngineType.Pool, ap=qb32_d[:, :])
            s32 = cp.tile_from(forced_dma_engine=mybir.EngineType.Pool, ap=s32_d[:, :])
            wcol = cp.tile_from(forced_dma_engine=mybir.EngineType.Pool, ap=wcol_d[:, :])
            mb1 = cp.tile_from(forced_dma_engine=mybir.EngineType.Pool, ap=mb1_d[:, :])
            ab1 = cp.tile_from(forced_dma_engine=mybir.EngineType.Pool, ap=ab1_d[:, :])
            ab2 = cp.tile_from(forced_dma_engine=mybir.EngineType.Pool, ap=ab2_d[:, :])

            # live tensors per quad within a super-group
            xb = [None] * SGQ     # bf16 input (step-1 matmul operand)
            u1 = [None] * SGQ     # fp32 upd1 + ab2 (step-2 residual base)
            c1b = [None] * SGQ    # bf16 step-2 matmul operand (xb + u1)
            smixT = [None] * SGQ  # bf16 mix output, T-layout

            hRs = [None] * SGQ

            def frontA(q, step, i):
                if step == 0:
                    xb[i] = fp.tile([D, QCOLS], bf, tag="xb", name=f"xb{i}")
                    nc.gpsimd.dma_start(xb[i][:], xb_d[q])
                cur_b = xb[i] if step == 0 else c1b[i]
                h = wp.tile([D, QCOLS], bf, tag="h")
                for j in range(QB):
                    cs = slice(j * GRP, (j + 1) * GRP)
                    hp = pp.tile([D, GRP], f32, tag="hp", bufs=3)
                    nc.tensor.matmul(
                        hp[:], w_m1[:], cur_b[:, cs], start=True, stop=True
                    )
                    if j % 2 == 0:
                        nc.scalar.activation(h[:, cs], hp[:], relu, bias=mb1[:])
                    else:
                        nc.vector.tensor_scalar(
                            h[:, cs], hp[:], mb1[:], 0.0, add, amax
                        )
                # NB: ALL transposes ride ONE DMA queue (nc.scalar) carrying
                # no DMACopy traffic: mixing kinds on a queue, or running
                # transposes on two queues concurrently, corrupts data via
                # the shared XBAR state. Copies go via nc.sync only.
                hRs[i] = wp.tile([D, QB * 4, D], bf, tag="hR", bufs=3,
                                 name=f"hR{i}")
                nc.sync.dma_start_transpose(hRs[i][:], h[:])

            def frontB(q, step, i):
                hR = hRs[i]
                smix = wp.tile([D, QCOLS], bf, tag="smix")
                for j in range(QB):
                    g = q * QB + j
                    cs = slice(j * GRP, (j + 1) * GRP)
                    mixp = pp.tile([D, GRP], f32, tag="mixp", bufs=3)
                    nc.tensor.matmul(
                        mixp[:], w_mx[:], hR[:, j * 4 : (j + 1) * 4, :],
                        start=True, stop=True,
                    )
                    # w-scale evac: one op, 0-stride broadcast of 4 w-columns
                    nc.vector.tensor_tensor(
                        smix[:, cs].rearrange("a (b c) -> a b c", b=4),
                        mixp[:].rearrange("a (b c) -> a b c", b=4),
                        wcol[:, 4 * g : 4 * g + 4].broadcast_to((D, 4, D)),
                        mult,
                    )
                smixT[i] = fp.tile([D, QB * 4, D], bf, tag="smixT", name=f"sT{i}")
                half = QCOLS // 2
                nc.sync.dma_start_transpose(
                    smixT[i][:, : QB * 2, :], smix[:, :half]
                )
                nc.sync.dma_start_transpose(
                    smixT[i][:, QB * 2 :, :], smix[:, half:]
                )

            def back(q, step, i):
                cur_b = xb[i] if step == 0 else c1b[i]
                if step == NSTEPS - 1:
                    onew = wp.tile([D, QCOLS], f32, tag="onew", name="onew")
                else:
                    u1[i] = fp.tile([D, QCOLS], f32, tag="u1", name=f"u1{i}")
                    c1b[i] = fp.tile([D, QCOLS], bf, tag="c1b", name=f"c1b{i}")
                for j in range(QB):
                    g = q * QB + j
                    cs = slice(j * GRP, (j + 1) * GRP)
                    gp = pp.tile([D, GRP], f32, tag="gp", bufs=1)
                    nc.tensor.matmul(
                        gp[:], w_q[:], smixT[i][:, j * 4 : (j + 1) * 4, :],
                        start=True, stop=False,
                    )
                    nc.tensor.matmul(
                        gp[:], w_a1t[:], cur_b[:, cs], start=False, stop=False
                    )
                    r = g % 32
                    fo = (g // 32) * GRP
                    nc.tensor.matmul(
                        gp[:],
                        qb32[0:32, r * D : (r + 1) * D],
                        s32[0:32, fo : fo + GRP],
                        start=False, stop=True,
                    )
                    gt = wp.tile([D, GRP], bf, tag="gt")
                    nc.scalar.activation(gt[:], gp[:], relu, bias=ab1[:])
                    up = pp.tile([D, GRP], f32, tag="up", bufs=1)
                    if step < NSTEPS - 1:
                        nc.tensor.matmul(
                            up[:], w_a2[:], gt[:], start=True, stop=True
                        )
                        # u1 = up1 + ab2 (fp32, step-2 residual base)
                        nc.scalar.activation(
                            u1[i][:, cs], up[:],
                            mybir.ActivationFunctionType.Identity, bias=ab2[:],
                        )
                        # step-2 matmul operand: bf16(xb + u1)
                        nc.vector.tensor_tensor(
                            c1b[i][:, cs], u1[i][:, cs], xb[i][:, cs], add
                        )
                    else:
                        nc.tensor.matmul(
                            up[:], w_a2[:], gt[:], start=True, stop=True
                        )
                        # device out = up1' + up2; host adds x + ab2
                        nc.vector.tensor_tensor(
                            onew[:, cs], up[:], u1[i][:, cs], add
                        )
                if step == NSTEPS - 1:
                    nc.gpsimd.dma_start(out_d[q], onew[:])

            for sq in range(nsq):
                qs = [q for q in range(sq * SGQ, min((sq + 1) * SGQ, nq))]
                for step in range(NSTEPS):
                    nq_s = len(qs)
                    lag = 2 if nq_s > 2 else 1
                    for k in range(nq_s + lag):
                        if k < nq_s:
                            frontA(qs[k], step, k)
                        if k >= lag:
                            frontB(qs[k - lag], step, k - lag)
                    for q in qs:
                        back(q, step, q - sq * SGQ)

    nc.compile()
    _nc_cache[cols] = nc
    return nc


def host_prep(module_states, connection_matrix, module_weights,
              mW1, mb1, mW2, mb2, aW1, ab1, aW2, ab2, ncores=NCORES):
    """Shard + precompute all host-side tensors. Returns (cols, in_maps)."""
    ms = np.asarray(module_states, np.float32)
    C = np.asarray(connection_matrix, np.float32)
    w = np.asarray(module_weights, np.float32)
    mW1 = np.asarray(mW1, np.float32)
    mb1 = np.asarray(mb1, np.float32)
    mW2 = np.asarray(mW2, np.float32)
    mb2 = np.asarray(mb2, np.float32)
    aW1 = np.asarray(aW1, np.float32)
    ab1 = np.asarray(ab1, np.float32)
    aW2 = np.asarray(aW2, np.float32)
    ab2 = np.asarray(ab2, np.float32)

    B = ms.shape[0]
    bsh = B // ncores
    cols = bsh * M
    ng = cols // GRP
    nq = cols // QCOLS
    sfree = ((ng + 31) // 32) * GRP

    rowmix = C.sum(axis=1)                      # [8], bias mix per module
    qb = mb2 @ aW1[D:, :]                       # [128]
    # qb selector table: row r of block r holds qb, so lhsT=qb32[0:32, r*128:..]
    # with rhs=s32[0:32, ...] picks out s-row r (the rest contract with zeros).
    qb32 = np.zeros((D, 32 * D), np.float32)
    for r in range(32):
        qb32[r, r * D : (r + 1) * D] = qb

    consts = {
        "wm1": mW1.astype(BF16),
        "wmx": np.kron(np.eye(16, dtype=np.float32), C.T).astype(BF16),
        "wq": (mW2 @ aW1[D:, :]).astype(BF16),
        "wa1t": np.ascontiguousarray(aW1[:D, :]).astype(BF16),
        "wa2": aW2.astype(BF16),
        "qb32": qb32.astype(BF16),
        "mb1": np.ascontiguousarray(mb1.reshape(D, 1)),
        "ab1": np.ascontiguousarray(ab1.reshape(D, 1)),
        "ab2": np.ascontiguousarray(ab2.reshape(D, 1)),
    }

    in_maps = []
    for k in range(ncores):
        shard = ms[k * bsh : (k + 1) * bsh]
        xT = shard.reshape(cols, D).T                       # [128, cols]
        xb = np.ascontiguousarray(
            xT.reshape(D, nq, QCOLS).transpose(1, 0, 2)     # [nq, 128, 2048]
        ).astype(BF16)
        wk = w[k * bsh : (k + 1) * bsh]
        wflat = wk.reshape(cols)
        wcol = np.ascontiguousarray(wflat.reshape(4 * ng, D).T)
        s = (wk * rowmix[None, :]).reshape(cols)
        s32 = np.zeros((D, sfree), BF16)
        for g in range(ng):
            s32[g % 32, (g // 32) * GRP : (g // 32 + 1) * GRP] = s[
                g * GRP : (g + 1) * GRP
            ].astype(BF16)
        in_maps.append({"xb": xb, "wcol": wcol, "s32": s32, **consts})
    return cols, in_maps


def gather_out(results, ab2, module_states=None, ncores=NCORES):
    ab2 = np.asarray(ab2, np.float32)
    outs = []
    for k in range(ncores):
        o = np.asarray(results[k]["out"])          # [nq, 128, 2048]
        nq = o.shape[0]
        cols = nq * QCOLS
        bsh = cols // M
        oT = o.transpose(1, 0, 2).reshape(D, cols)  # [128, cols]
        # device out = up1 + ab2 + up2; add x and the final step's ab2
        o = oT.T.reshape(bsh, M, D) + ab2[None, None, :]
        outs.append(o)
    out = np.concatenate(outs, 0)
    out += np.asarray(module_states, np.float32)
    return out.astype(np.float32)


def _run(inputs, trace=False):
    from concourse.bass_utils import run_bass_kernel_spmd

    cols, in_maps = host_prep(**inputs)
    nc = build_nc(cols)
    res = run_bass_kernel_spmd(nc, in_maps, list(range(NCORES)), trace=trace)
    out = gather_out(res.results, inputs["ab2"], inputs["module_states"])
    return out, res


def kernel(**inputs):
    out, _ = _run(inputs, trace=False)
    return out


# revision 29
# speedup vs baseline: 1.1440x; 1.1440x over previous
"""Trainium2 Bass kernel for nn_MessagePassing (gnn_message_passing).

Reference computation (2 steps):
    h    = relu(cur @ mW1 + mb1)                      # per-module MLP layer 1
    msg  = h @ mW2 + mb2                              # per-module MLP layer 2
    rec  = einsum('mn,bnd->bmd', C, msg) * w[:,:,None]
    g    = relu(concat([cur, rec], -1) @ aW1 + ab1)
    cur  = cur + g @ aW2 + ab2

Strategy (data-parallel over 8 NeuronCores, 8192 batch rows each):
  * T-layout on chip: SBUF tiles are [128 features (partitions), cols] where
    a column is one (b, m) row of the flattened [B*M, 128] stream. Every
    per-module linear layer is one weights-stationary bf16 matmul
    (out = W.T @ x_T) streaming 512 columns per instruction; PSUM fp32;
    the fp32 residual stream never passes through bf16.
  * Algebraic refactor (host-side): row scaling by w commutes with
    right-matmuls and the 8x8 mix commutes with feature transforms:
        rec_contrib = (w ⊙ mix(h)) @ Q + s ⊗ qb
    with Q = mW2 @ aW1_bot, qb = mb2 @ aW1_bot, s = w * rowsum(C). This
    eliminates the mW2 pass. The rank-1 s⊗qb term is one K=32 matmul per
    group against a host-built selector table. ab2 is deferred to the host
    (+NSTEPS*ab2 at the end); step-1's bf16 operand re-adds it on-device.
  * The module mix runs on the PE as ONE weights-stationary matmul with
    W_mix = kron(I_16, C.T) (block-diagonal over 16 b-groups x 8 modules).
    Rows<->partitions movement uses the DMA XBAR sub-tiled transpose
    ([128,(t,d)] <-> [d,(t,128)]), batched 4 groups (2048 cols) per DMA
    instruction to amortize the ~600ns trigger + ~20ns/16x128-tile
    descriptor cost.
  * DRAM I/O is quad-blocked ([nq, 128, 2048] contiguous 1MB blocks; the
    flat layout cost 4.1us of descriptor generation per DMA).
  * Engine balance per group-step: PE 6 matmuls; ACT h-relu evac + trB
    trigger; DVE g-relu evac + residual + w-scale; SP loads/stores + trA
    trigger; GPSIMD the step-1 bf16 recast. The w-scale is one
    tensor_tensor against a 0-stride broadcast of per-tile w columns.
  * Emission is software-pipelined over super-groups of 2 quads: all
    front-half work (h matmul, relu, trA, mix, w-scale, trB) for a quad is
    emitted before its back-half (aW1/rank1 accumulate, relu, aW2,
    residual), so the PE's in-order stream never waits on the transpose
    round-trip.
"""

import os
import sys

import numpy as np

try:
    import concourse.bass as bass
except ImportError:  # harness runs kernel.py from a bare directory
    sys.path.insert(0, "/opt/trn_rl_repo")
    import concourse.bass as bass

import ml_dtypes
import concourse.bacc as bacc
import concourse.mybir as mybir
from concourse.tile import TileContext

BF16 = ml_dtypes.bfloat16
D = 128
M = 8
GRP = 512
QB = 4                  # groups per quad (DMA/transpose batch)
QCOLS = QB * GRP        # 2048
SGQ = 6                 # quads per super-group (software pipeline depth)
NCORES = 8
NSTEPS = 2

_nc_cache = {}


def build_nc(cols):
    """Build (and cache) the per-core Bass program for a `cols`-wide shard."""
    if cols in _nc_cache:
        return _nc_cache[cols]
    assert cols % QCOLS == 0
    ng = cols // GRP
    nq = cols // QCOLS
    sfree = ((ng + 31) // 32) * GRP

    f32 = mybir.dt.float32
    bf = mybir.dt.bfloat16
    relu = mybir.ActivationFunctionType.Relu
    add = mybir.AluOpType.add
    mult = mybir.AluOpType.mult
    amax = mybir.AluOpType.max

    nc = bacc.Bacc(trn_type="TRN2")
    xb_d = nc.declare_dram_parameter("xb", [nq, D, QCOLS], bf, isOutput=False)
    wcol_d = nc.declare_dram_parameter("wcol", [D, 4 * ng], f32, isOutput=False)
    s32_d = nc.declare_dram_parameter("s32", [D, sfree], bf, isOutput=False)
    qb32_d = nc.declare_dram_parameter("qb32", [D, 32 * D], bf, isOutput=False)
    wm1_d = nc.declare_dram_parameter("wm1", [D, D], bf, isOutput=False)
    wmx_d = nc.declare_dram_parameter("wmx", [D, D], bf, isOutput=False)
    wq_d = nc.declare_dram_parameter("wq", [D, D], bf, isOutput=False)
    wa1t_d = nc.declare_dram_parameter("wa1t", [D, D], bf, isOutput=False)
    wa2_d = nc.declare_dram_parameter("wa2", [D, D], bf, isOutput=False)
    mb1_d = nc.declare_dram_parameter("mb1", [D, 1], f32, isOutput=False)
    ab1_d = nc.declare_dram_parameter("ab1", [D, 1], f32, isOutput=False)
    ab2_d = nc.declare_dram_parameter("ab2", [D, 1], f32, isOutput=False)
    out_d = nc.declare_dram_parameter("out", [nq, D, QCOLS], f32, isOutput=True)

    nsq = (nq + SGQ - 1) // SGQ

    with TileContext(nc) as tc:
        with (
            tc.tile_pool(name="consts", bufs=1) as cp,
            tc.tile_pool(name="work", bufs=2) as wp,
            tc.tile_pool(name="pipe", bufs=SGQ + 1) as fp,
            tc.tile_pool(name="psum", bufs=2, space="PSUM") as pp,
        ):
            w_m1 = cp.tile_from(forced_dma_engine=mybir.EngineType.Pool, ap=wm1_d[:, :])
            w_mx = cp.tile_from(forced_dma_engine=mybir.EngineType.Pool, ap=wmx_d[:, :])
            w_q = cp.tile_from(forced_dma_engine=mybir.EngineType.Pool, ap=wq_d[:, :])
            w_a1t = cp.tile_from(forced_dma_engine=mybir.EngineType.Pool, ap=wa1t_d[:, :])
            w_a2 = cp.tile_from(forced_dma_engine=mybir.EngineType.Pool, ap=wa2_d[:, :])
            qb32 = cp.tile_from(forced_dma_engine=mybir.EngineType.Pool, ap=qb32_d[:, :])
            s32 = cp.tile_from(forced_dma_engine=mybir.EngineType.Pool, ap=s32_d[:, :])
            wcol = cp.tile_from(forced_dma_engine=mybir.EngineType.Pool, ap=wcol_d[:, :])
            mb1 = cp.tile_from(forced_dma_engine=mybir.EngineType.Pool, ap=mb1_d[:, :])
            ab1 = cp.tile_from(forced_dma_engine=mybir.EngineType.Pool, ap=ab1_d[:, :])
            ab2 = cp.tile_from(forced_dma_engine=mybir.EngineType.Pool, ap=ab2_d[:, :])

            # live tensors per quad within a super-group
            xb = [None] * SGQ     # bf16 input (step-1 matmul operand)
            u1 = [None] * SGQ     # fp32 upd1 + ab2 (step-2 residual base)
            c1b = [None] * SGQ    # bf16 step-2 matmul operand (xb + u1)
            smixT = [None] * SGQ  # bf16 mix output, T-layout

            hRs = [None] * SGQ

            def frontA(q, step, i):
                if step == 0:
                    xb[i] = fp.tile([D, QCOLS], bf, tag="xb", name=f"xb{i}")
                    nc.gpsimd.dma_start(xb[i][:], xb_d[q])
                cur_b = xb[i] if step == 0 else c1b[i]
                h = wp.tile([D, QCOLS], bf, tag="h")
                for j in range(QB):
                    cs = slice(j * GRP, (j + 1) * GRP)
                    hp = pp.tile([D, GRP], f32, tag="hp")
                    nc.tensor.matmul(
                        hp[:], w_m1[:], cur_b[:, cs], start=True, stop=True
                    )
                    if j % 2 == 0:
                        nc.scalar.activation(h[:, cs], hp[:], relu, bias=mb1[:])
                    else:
                        nc.vector.tensor_scalar(
                            h[:, cs], hp[:], mb1[:], 0.0, add, amax
                        )
                # NB: ALL transposes ride ONE DMA queue (nc.scalar) carrying
                # no DMACopy traffic: mixing kinds on a queue, or running
                # transposes on two queues concurrently, corrupts data via
                # the shared XBAR state. Copies go via nc.sync only.
                hRs[i] = wp.tile([D, QB * 4, D], bf, tag="hR", bufs=3,
                                 name=f"hR{i}")
                nc.sync.dma_start_transpose(hRs[i][:], h[:])

            def frontB(q, step, i):
                hR = hRs[i]
                smix = wp.tile([D, QCOLS], bf, tag="smix")
                for j in range(QB):
                    g = q * QB + j
                    cs = slice(j * GRP, (j + 1) * GRP)
                    mixp = pp.tile([D, GRP], f32, tag="mixp")
                    nc.tensor.matmul(
                        mixp[:], w_mx[:], hR[:, j * 4 : (j + 1) * 4, :],
                        start=True, stop=True,
                    )
                    # w-scale evac: one op, 0-stride broadcast of 4 w-columns
                    nc.vector.tensor_tensor(
                        smix[:, cs].rearrange("a (b c) -> a b c", b=4),
                        mixp[:].rearrange("a (b c) -> a b c", b=4),
                        wcol[:, 4 * g : 4 * g + 4].broadcast_to((D, 4, D)),
                        mult,
                    )
                smixT[i] = fp.tile([D, QB * 4, D], bf, tag="smixT", name=f"sT{i}")
                half = QCOLS // 2
                nc.sync.dma_start_transpose(
                    smixT[i][:, : QB * 2, :], smix[:, :half]
                )
                nc.sync.dma_start_transpose(
                    smixT[i][:, QB * 2 :, :], smix[:, half:]
                )

            def back(q, step, i):
                cur_b = xb[i] if step == 0 else c1b[i]
                if step == NSTEPS - 1:
                    onew = wp.tile([D, QCOLS], f32, tag="onew", name="onew")
                else:
                    u1[i] = fp.tile([D, QCOLS], f32, tag="u1", name=f"u1{i}")
                    c1b[i] = fp.tile([D, QCOLS], bf, tag="c1b", name=f"c1b{i}")
                for j in range(QB):
                    g = q * QB + j
                    cs = slice(j * GRP, (j + 1) * GRP)
                    gp = pp.tile([D, GRP], f32, tag="gp")
                    nc.tensor.matmul(
                        gp[:], w_q[:], smixT[i][:, j * 4 : (j + 1) * 4, :],
                        start=True, stop=False,
                    )
                    nc.tensor.matmul(
                        gp[:], w_a1t[:], cur_b[:, cs], start=False, stop=False
                    )
                    r = g % 32
                    fo = (g // 32) * GRP
                    nc.tensor.matmul(
                        gp[:],
                        qb32[0:32, r * D : (r + 1) * D],
                        s32[0:32, fo : fo + GRP],
                        start=False, stop=True,
                    )
                    gt = wp.tile([D, GRP], bf, tag="gt")
                    nc.scalar.activation(gt[:], gp[:], relu, bias=ab1[:])
                    up = pp.tile([D, GRP], f32, tag="up")
                    if step < NSTEPS - 1:
                        nc.tensor.matmul(
                            up[:], w_a2[:], gt[:], start=True, stop=True
                        )
                        # u1 = up1 + ab2 (fp32, step-2 residual base)
                        nc.scalar.activation(
                            u1[i][:, cs], up[:],
                            mybir.ActivationFunctionType.Identity, bias=ab2[:],
                        )
                        # step-2 matmul operand: bf16(xb + u1)
                        nc.vector.tensor_tensor(
                            c1b[i][:, cs], u1[i][:, cs], xb[i][:, cs], add
                        )
                    else:
                        nc.tensor.matmul(
                            up[:], w_a2[:], gt[:], start=True, stop=True
                        )
                        # device out = up1' + up2; host adds x + ab2
                        nc.vector.tensor_tensor(
                            onew[:, cs], up[:], u1[i][:, cs], add
                        )
                if step == NSTEPS - 1:
                    nc.gpsimd.dma_start(out_d[q], onew[:])

            for sq in range(nsq):
                qs = [q for q in range(sq * SGQ, min((sq + 1) * SGQ, nq))]
                for step in range(NSTEPS):
                    nq_s = len(qs)
                    lag = 2 if nq_s > 2 else 1
                    for k in range(nq_s + lag):
                        if k < nq_s:
                            frontA(qs[k], step, k)
                        if k >= lag:
                            frontB(qs[k - lag], step, k - lag)
                    for q in qs:
                        back(q, step, q - sq * SGQ)

    nc.compile()
    _nc_cache[cols] = nc
    return nc


def host_prep(module_states, connection_matrix, module_weights,
              mW1, mb1, mW2, mb2, aW1, ab1, aW2, ab2, ncores=NCORES):
    """Shard + precompute all host-side tensors. Returns (cols, in_maps)."""
    ms = np.asarray(module_states, np.float32)
    C = np.asarray(connection_matrix, np.float32)
    w = np.asarray(module_weights, np.float32)
    mW1 = np.asarray(mW1, np.float32)
    mb1 = np.asarray(mb1, np.float32)
    mW2 = np.asarray(mW2, np.float32)
    mb2 = np.asarray(mb2, np.float32)
    aW1 = np.asarray(aW1, np.float32)
    ab1 = np.asarray(ab1, np.float32)
    aW2 = np.asarray(aW2, np.float32)
    ab2 = np.asarray(ab2, np.float32)

    B = ms.shape[0]
    bsh = B // ncores
    cols = bsh * M
    ng = cols // GRP
    nq = cols // QCOLS
    sfree = ((ng + 31) // 32) * GRP

    rowmix = C.sum(axis=1)                      # [8], bias mix per module
    qb = mb2 @ aW1[D:, :]                       # [128]
    # qb selector table: row r of block r holds qb, so lhsT=qb32[0:32, r*128:..]
    # with rhs=s32[0:32, ...] picks out s-row r (the rest contract with zeros).
    qb32 = np.zeros((D, 32 * D), np.float32)
    for r in range(32):
        qb32[r, r * D : (r + 1) * D] = qb

    consts = {
        "wm1": mW1.astype(BF16),
        "wmx": np.kron(np.eye(16, dtype=np.float32), C.T).astype(BF16),
        "wq": (mW2 @ aW1[D:, :]).astype(BF16),
        "wa1t": np.ascontiguousarray(aW1[:D, :]).astype(BF16),
        "wa2": aW2.astype(BF16),
        "qb32": qb32.astype(BF16),
        "mb1": np.ascontiguousarray(mb1.reshape(D, 1)),
        "ab1": np.ascontiguousarray(ab1.reshape(D, 1)),
        "ab2": np.ascontiguousarray(ab2.reshape(D, 1)),
    }

    in_maps = []
    for k in range(ncores):
        shard = ms[k * bsh : (k + 1) * bsh]
        xT = shard.reshape(cols, D).T                       # [128, cols]
        xb = np.ascontiguousarray(
            xT.reshape(D, nq, QCOLS).transpose(1, 0, 2)     # [nq, 128, 2048]
        ).astype(BF16)
        wk = w[k * bsh : (k + 1) * bsh]
        wflat = wk.reshape(cols)
        wcol = np.ascontiguousarray(wflat.reshape(4 * ng, D).T)
        s = (wk * rowmix[None, :]).reshape(cols)
        s32 = np.zeros((D, sfree), BF16)
        for g in range(ng):
            s32[g % 32, (g // 32) * GRP : (g // 32 + 1) * GRP] = s[
                g * GRP : (g + 1) * GRP
            ].astype(BF16)
        in_maps.append({"xb": xb, "wcol": wcol, "s32": s32, **consts})
    return cols, in_maps


def gather_out(results, ab2, module_states=None, ncores=NCORES):
    ab2 = np.asarray(ab2, np.float32)
    outs = []
    for k in range(ncores):
        o = np.asarray(results[k]["out"])          # [nq, 128, 2048]
        nq = o.shape[0]
        cols = nq * QCOLS
        bsh = cols // M
        oT = o.transpose(1, 0, 2).reshape(D, cols)  # [128, cols]
        # device out = up1 + ab2 + up2; add x and the final step's ab2
        o = oT.T.reshape(bsh, M, D) + ab2[None, None, :]
        outs.append(o)
    out = np.concatenate(outs, 0)
    out += np.asarray(module_states, np.float32)
    return out.astype(np.float32)


def _run(inputs, trace=False):
    from concourse.bass_utils import run_bass_kernel_spmd

    cols, in_maps = host_prep(**inputs)
    nc = build_nc(cols)
    res = run_bass_kernel_spmd(nc, in_maps, list(range(NCORES)), trace=trace)
    out = gather_out(res.results, inputs["ab2"], inputs["module_states"])
    return out, res


def kernel(**inputs):
    out, _ = _run(inputs, trace=False)
    return out


# revision 30
# speedup vs baseline: 1.2226x; 1.0688x over previous
"""Trainium2 Bass kernel for nn_MessagePassing (gnn_message_passing).

Reference computation (2 steps):
    h    = relu(cur @ mW1 + mb1)                      # per-module MLP layer 1
    msg  = h @ mW2 + mb2                              # per-module MLP layer 2
    rec  = einsum('mn,bnd->bmd', C, msg) * w[:,:,None]
    g    = relu(concat([cur, rec], -1) @ aW1 + ab1)
    cur  = cur + g @ aW2 + ab2

Strategy (data-parallel over 8 NeuronCores, 8192 batch rows each):
  * T-layout on chip: SBUF tiles are [128 features (partitions), cols] where
    a column is one (b, m) row of the flattened [B*M, 128] stream. Every
    per-module linear layer is one weights-stationary bf16 matmul
    (out = W.T @ x_T) streaming 512 columns per instruction; PSUM fp32;
    the fp32 residual stream never passes through bf16.
  * Algebraic refactor (host-side): row scaling by w commutes with
    right-matmuls and the 8x8 mix commutes with feature transforms:
        rec_contrib = (w ⊙ mix(h)) @ Q + s ⊗ qb
    with Q = mW2 @ aW1_bot, qb = mb2 @ aW1_bot, s = w * rowsum(C). This
    eliminates the mW2 pass. The rank-1 s⊗qb term is one K=32 matmul per
    group against a host-built selector table. ab2 is deferred to the host
    (+NSTEPS*ab2 at the end); step-1's bf16 operand re-adds it on-device.
  * The module mix runs on the PE as ONE weights-stationary matmul with
    W_mix = kron(I_16, C.T) (block-diagonal over 16 b-groups x 8 modules).
    Rows<->partitions movement uses the DMA XBAR sub-tiled transpose
    ([128,(t,d)] <-> [d,(t,128)]), batched 4 groups (2048 cols) per DMA
    instruction to amortize the ~600ns trigger + ~20ns/16x128-tile
    descriptor cost.
  * DRAM I/O is quad-blocked ([nq, 128, 2048] contiguous 1MB blocks; the
    flat layout cost 4.1us of descriptor generation per DMA).
  * Engine balance per group-step: PE 6 matmuls; ACT h-relu evac + trB
    trigger; DVE g-relu evac + residual + w-scale; SP loads/stores + trA
    trigger; GPSIMD the step-1 bf16 recast. The w-scale is one
    tensor_tensor against a 0-stride broadcast of per-tile w columns.
  * Emission is software-pipelined over super-groups of 2 quads: all
    front-half work (h matmul, relu, trA, mix, w-scale, trB) for a quad is
    emitted before its back-half (aW1/rank1 accumulate, relu, aW2,
    residual), so the PE's in-order stream never waits on the transpose
    round-trip.
"""

import os
import sys

import numpy as np

try:
    import concourse.bass as bass
except ImportError:  # harness runs kernel.py from a bare directory
    sys.path.insert(0, "/opt/trn_rl_repo")
    import concourse.bass as bass

import ml_dtypes
import concourse.bacc as bacc
import concourse.mybir as mybir
from concourse.tile import TileContext

BF16 = ml_dtypes.bfloat16
D = 128
M = 8
GRP = 512
QB = 4                  # groups per quad (DMA/transpose batch)
QCOLS = QB * GRP        # 2048
SGQ = 6                 # quads per super-group (software pipeline depth)
NCORES = 8
NSTEPS = 2

_nc_cache = {}


def build_nc(cols):
    """Build (and cache) the per-core Bass program for a `cols`-wide shard."""
    if cols in _nc_cache:
        return _nc_cache[cols]
    assert cols % QCOLS == 0
    ng = cols // GRP
    nq = cols // QCOLS
    sfree = ((ng + 31) // 32) * GRP

    f32 = mybir.dt.float32
    bf = mybir.dt.bfloat16
    relu = mybir.ActivationFunctionType.Relu
    add = mybir.AluOpType.add
    mult = mybir.AluOpType.mult
    amax = mybir.AluOpType.max

    nc = bacc.Bacc(trn_type="TRN2")
    xb_d = nc.declare_dram_parameter("xb", [nq, D, QCOLS], bf, isOutput=False)
    wcol_d = nc.declare_dram_parameter("wcol", [D, 4 * ng], f32, isOutput=False)
    s32_d = nc.declare_dram_parameter("s32", [D, sfree], bf, isOutput=False)
    qb32_d = nc.declare_dram_parameter("qb32", [D, 32 * D], bf, isOutput=False)
    wm1_d = nc.declare_dram_parameter("wm1", [D, D], bf, isOutput=False)
    wmx_d = nc.declare_dram_parameter("wmx", [D, D], bf, isOutput=False)
    wq_d = nc.declare_dram_parameter("wq", [D, D], bf, isOutput=False)
    wa1t_d = nc.declare_dram_parameter("wa1t", [D, D], bf, isOutput=False)
    wa2_d = nc.declare_dram_parameter("wa2", [D, D], bf, isOutput=False)
    mb1_d = nc.declare_dram_parameter("mb1", [D, 1], f32, isOutput=False)
    ab1_d = nc.declare_dram_parameter("ab1", [D, 1], f32, isOutput=False)
    ab2_d = nc.declare_dram_parameter("ab2", [D, 1], f32, isOutput=False)
    out_d = nc.declare_dram_parameter("out", [nq, D, QCOLS], bf, isOutput=True)

    nsq = (nq + SGQ - 1) // SGQ

    with TileContext(nc) as tc:
        with (
            tc.tile_pool(name="consts", bufs=1) as cp,
            tc.tile_pool(name="work", bufs=2) as wp,
            tc.tile_pool(name="pipe", bufs=SGQ + 1) as fp,
            tc.tile_pool(name="psum", bufs=2, space="PSUM") as pp,
        ):
            w_m1 = cp.tile_from(forced_dma_engine=mybir.EngineType.Pool, ap=wm1_d[:, :])
            w_mx = cp.tile_from(forced_dma_engine=mybir.EngineType.Pool, ap=wmx_d[:, :])
            w_q = cp.tile_from(forced_dma_engine=mybir.EngineType.Pool, ap=wq_d[:, :])
            w_a1t = cp.tile_from(forced_dma_engine=mybir.EngineType.Pool, ap=wa1t_d[:, :])
            w_a2 = cp.tile_from(forced_dma_engine=mybir.EngineType.Pool, ap=wa2_d[:, :])
            qb32 = cp.tile_from(forced_dma_engine=mybir.EngineType.Pool, ap=qb32_d[:, :])
            s32 = cp.tile_from(forced_dma_engine=mybir.EngineType.Pool, ap=s32_d[:, :])
            wcol = cp.tile_from(forced_dma_engine=mybir.EngineType.Pool, ap=wcol_d[:, :])
            mb1 = cp.tile_from(forced_dma_engine=mybir.EngineType.Pool, ap=mb1_d[:, :])
            ab1 = cp.tile_from(forced_dma_engine=mybir.EngineType.Pool, ap=ab1_d[:, :])
            ab2 = cp.tile_from(forced_dma_engine=mybir.EngineType.Pool, ap=ab2_d[:, :])

            # live tensors per quad within a super-group
            xb = [None] * SGQ     # bf16 input (step-1 matmul operand)
            u1 = [None] * SGQ     # fp32 upd1 + ab2 (step-2 residual base)
            c1b = [None] * SGQ    # bf16 step-2 matmul operand (xb + u1)
            smixT = [None] * SGQ  # bf16 mix output, T-layout

            hRs = [None] * SGQ

            def frontA(q, step, i):
                if step == 0:
                    xb[i] = fp.tile([D, QCOLS], bf, tag="xb", name=f"xb{i}")
                    nc.gpsimd.dma_start(xb[i][:], xb_d[q])
                cur_b = xb[i] if step == 0 else c1b[i]
                h = wp.tile([D, QCOLS], bf, tag="h")
                for j in range(QB):
                    cs = slice(j * GRP, (j + 1) * GRP)
                    hp = pp.tile([D, GRP], f32, tag="hp")
                    nc.tensor.matmul(
                        hp[:], w_m1[:], cur_b[:, cs], start=True, stop=True
                    )
                    if j % 2 == 0:
                        nc.scalar.activation(h[:, cs], hp[:], relu, bias=mb1[:])
                    else:
                        nc.vector.tensor_scalar(
                            h[:, cs], hp[:], mb1[:], 0.0, add, amax
                        )
                # NB: ALL transposes ride ONE DMA queue (nc.scalar) carrying
                # no DMACopy traffic: mixing kinds on a queue, or running
                # transposes on two queues concurrently, corrupts data via
                # the shared XBAR state. Copies go via nc.sync only.
                hRs[i] = wp.tile([D, QB * 4, D], bf, tag="hR", bufs=3,
                                 name=f"hR{i}")
                nc.sync.dma_start_transpose(hRs[i][:], h[:])

            def frontB(q, step, i):
                hR = hRs[i]
                smix = wp.tile([D, QCOLS], bf, tag="smix")
                for j in range(QB):
                    g = q * QB + j
                    cs = slice(j * GRP, (j + 1) * GRP)
                    mixp = pp.tile([D, GRP], f32, tag="mixp")
                    nc.tensor.matmul(
                        mixp[:], w_mx[:], hR[:, j * 4 : (j + 1) * 4, :],
                        start=True, stop=True,
                    )
                    # w-scale evac: one op, 0-stride broadcast of 4 w-columns
                    nc.vector.tensor_tensor(
                        smix[:, cs].rearrange("a (b c) -> a b c", b=4),
                        mixp[:].rearrange("a (b c) -> a b c", b=4),
                        wcol[:, 4 * g : 4 * g + 4].broadcast_to((D, 4, D)),
                        mult,
                    )
                smixT[i] = fp.tile([D, QB * 4, D], bf, tag="smixT", name=f"sT{i}")
                half = QCOLS // 2
                nc.sync.dma_start_transpose(
                    smixT[i][:, : QB * 2, :], smix[:, :half]
                )
                nc.sync.dma_start_transpose(
                    smixT[i][:, QB * 2 :, :], smix[:, half:]
                )

            def back(q, step, i):
                cur_b = xb[i] if step == 0 else c1b[i]
                if step == NSTEPS - 1:
                    onew = wp.tile([D, QCOLS], bf, tag="onew", bufs=3, name="onew")
                else:
                    u1[i] = fp.tile([D, QCOLS], f32, tag="u1", name=f"u1{i}")
                    c1b[i] = fp.tile([D, QCOLS], bf, tag="c1b", name=f"c1b{i}")
                for j in range(QB):
                    g = q * QB + j
                    cs = slice(j * GRP, (j + 1) * GRP)
                    gp = pp.tile([D, GRP], f32, tag="gp")
                    nc.tensor.matmul(
                        gp[:], w_q[:], smixT[i][:, j * 4 : (j + 1) * 4, :],
                        start=True, stop=False,
                    )
                    nc.tensor.matmul(
                        gp[:], w_a1t[:], cur_b[:, cs], start=False, stop=False
                    )
                    r = g % 32
                    fo = (g // 32) * GRP
                    nc.tensor.matmul(
                        gp[:],
                        qb32[0:32, r * D : (r + 1) * D],
                        s32[0:32, fo : fo + GRP],
                        start=False, stop=True,
                    )
                    gt = wp.tile([D, GRP], bf, tag="gt")
                    nc.scalar.activation(gt[:], gp[:], relu, bias=ab1[:])
                    up = pp.tile([D, GRP], f32, tag="up")
                    if step < NSTEPS - 1:
                        nc.tensor.matmul(
                            up[:], w_a2[:], gt[:], start=True, stop=True
                        )
                        # u1 = up1 + ab2 (fp32, step-2 residual base)
                        nc.scalar.activation(
                            u1[i][:, cs], up[:],
                            mybir.ActivationFunctionType.Identity, bias=ab2[:],
                        )
                        # step-2 matmul operand: bf16(xb + u1)
                        nc.vector.tensor_tensor(
                            c1b[i][:, cs], u1[i][:, cs], xb[i][:, cs], add
                        )
                    else:
                        nc.tensor.matmul(
                            up[:], w_a2[:], gt[:], start=True, stop=True
                        )
                        # device out = up1' + up2; host adds x + ab2
                        nc.vector.tensor_tensor(
                            onew[:, cs], up[:], u1[i][:, cs], add
                        )
                if step == NSTEPS - 1:
                    nc.gpsimd.dma_start(out_d[q], onew[:])

            for sq in range(nsq):
                qs = [q for q in range(sq * SGQ, min((sq + 1) * SGQ, nq))]
                for step in range(NSTEPS):
                    nq_s = len(qs)
                    lag = 2 if nq_s > 2 else 1
                    for k in range(nq_s + lag):
                        if k < nq_s:
                            frontA(qs[k], step, k)
                        if k >= lag:
                            frontB(qs[k - lag], step, k - lag)
                    for q in qs:
                        back(q, step, q - sq * SGQ)

    nc.compile()
    _nc_cache[cols] = nc
    return nc


def host_prep(module_states, connection_matrix, module_weights,
              mW1, mb1, mW2, mb2, aW1, ab1, aW2, ab2, ncores=NCORES):
    """Shard + precompute all host-side tensors. Returns (cols, in_maps)."""
    ms = np.asarray(module_states, np.float32)
    C = np.asarray(connection_matrix, np.float32)
    w = np.asarray(module_weights, np.float32)
    mW1 = np.asarray(mW1, np.float32)
    mb1 = np.asarray(mb1, np.float32)
    mW2 = np.asarray(mW2, np.float32)
    mb2 = np.asarray(mb2, np.float32)
    aW1 = np.asarray(aW1, np.float32)
    ab1 = np.asarray(ab1, np.float32)
    aW2 = np.asarray(aW2, np.float32)
    ab2 = np.asarray(ab2, np.float32)

    B = ms.shape[0]
    bsh = B // ncores
    cols = bsh * M
    ng = cols // GRP
    nq = cols // QCOLS
    sfree = ((ng + 31) // 32) * GRP

    rowmix = C.sum(axis=1)                      # [8], bias mix per module
    qb = mb2 @ aW1[D:, :]                       # [128]
    # qb selector table: row r of block r holds qb, so lhsT=qb32[0:32, r*128:..]
    # with rhs=s32[0:32, ...] picks out s-row r (the rest contract with zeros).
    qb32 = np.zeros((D, 32 * D), np.float32)
    for r in range(32):
        qb32[r, r * D : (r + 1) * D] = qb

    consts = {
        "wm1": mW1.astype(BF16),
        "wmx": np.kron(np.eye(16, dtype=np.float32), C.T).astype(BF16),
        "wq": (mW2 @ aW1[D:, :]).astype(BF16),
        "wa1t": np.ascontiguousarray(aW1[:D, :]).astype(BF16),
        "wa2": aW2.astype(BF16),
        "qb32": qb32.astype(BF16),
        "mb1": np.ascontiguousarray(mb1.reshape(D, 1)),
        "ab1": np.ascontiguousarray(ab1.reshape(D, 1)),
        "ab2": np.ascontiguousarray(ab2.reshape(D, 1)),
    }

    in_maps = []
    for k in range(ncores):
        shard = ms[k * bsh : (k + 1) * bsh]
        xT = shard.reshape(cols, D).T                       # [128, cols]
        xb = np.ascontiguousarray(
            xT.reshape(D, nq, QCOLS).transpose(1, 0, 2)     # [nq, 128, 2048]
        ).astype(BF16)
        wk = w[k * bsh : (k + 1) * bsh]
        wflat = wk.reshape(cols)
        wcol = np.ascontiguousarray(wflat.reshape(4 * ng, D).T)
        s = (wk * rowmix[None, :]).reshape(cols)
        s32 = np.zeros((D, sfree), BF16)
        for g in range(ng):
            s32[g % 32, (g // 32) * GRP : (g // 32 + 1) * GRP] = s[
                g * GRP : (g + 1) * GRP
            ].astype(BF16)
        in_maps.append({"xb": xb, "wcol": wcol, "s32": s32, **consts})
    return cols, in_maps


def gather_out(results, ab2, module_states=None, ncores=NCORES):
    ab2 = np.asarray(ab2, np.float32)
    outs = []
    for k in range(ncores):
        o = np.asarray(results[k]["out"]).astype(np.float32)
        nq = o.shape[0]
        cols = nq * QCOLS
        bsh = cols // M
        oT = o.transpose(1, 0, 2).reshape(D, cols)  # [128, cols]
        # device out = up1 + ab2 + up2; add x and the final step's ab2
        o = oT.T.reshape(bsh, M, D) + ab2[None, None, :]
        outs.append(o)
    out = np.concatenate(outs, 0)
    out += np.asarray(module_states, np.float32)
    return out.astype(np.float32)


def _run(inputs, trace=False):
    from concourse.bass_utils import run_bass_kernel_spmd

    cols, in_maps = host_prep(**inputs)
    nc = build_nc(cols)
    res = run_bass_kernel_spmd(nc, in_maps, list(range(NCORES)), trace=trace)
    out = gather_out(res.results, inputs["ab2"], inputs["module_states"])
    return out, res


def kernel(**inputs):
    out, _ = _run(inputs, trace=False)
    return out
